# revision 101
# baseline (speedup 1.0000x reference)
"""MultiHeadAttention Trainium2 kernel (8 NeuronCores).

Reference computation (torch-style Linear, x @ W.T):
    k = key @ W_k.T; v = value @ W_v.T; q = query (no projection)
    scores = q @ k.T / sqrt(64) per head; attn = softmax(scores)
    out = (attn @ v) @ W_o.T

Sharding: core = (batch b, head-group g) with b in {0,1}, g in {0..3};
each core owns 4 heads of one batch. Projection weights are column-split
by head so K/V projections and attention stay core-local; the final W_o
matmul is computed as a partial sum over the core's 256 head-channels and
the 4 partials per batch are summed on host.

On-device dataflow per core:
    kT[128,2,4096] = W_k-halves.T @ keyT        (fp32r, contraction over embed)
    vx[...,65]     = valueT.T @ W_v (+ ones col) stored bf16
    scores[t,q]    = kT_h.T @ q_h               (fp32r, per head, 128-token
                                                 chunks x 512-query tiles)
    ex             = exp(scores/8 - 2.5) bf16   (softmax-invariant shift;
                     3/4 of chunks on ScalarE, 1/4 via a one-instruction
                     Schraudolph bit-trick on DVE: i16 = s*A + B -> bf16)
    oT[q,65]       = ex-chunk(stationary).T @ vx(moving)   (transposed
                     attn@v: 65-cycle matmuls; denominator lands in col 64
                     so normalization is a per-partition scalar)
    nm             = oT[:,:,0:64] * recip(oT[:,:,64]) bf16
    nmT            = PE transpose of nm (2 heads stacked -> 128-deep W_o
                     contraction)
    out_partial    = nmT.T @ W_o(bf16), 4 partials per batch summed on host
"""

import os
import numpy as np

import concourse.bacc as bacc
import concourse.tile as tile
import concourse.mybir as mybir
from concourse.bass_utils import run_bass_kernel_spmd

F32 = mybir.dt.float32
F32R = mybir.dt.float32r
BF16 = mybir.dt.bfloat16
I16 = mybir.dt.int16
EXPF = mybir.ActivationFunctionType.Exp

B, NQ, NK, E, H, D = 2, 2048, 4096, 1024, 16, 64
HPC = 4          # heads per core
C = HPC * D      # head-channels per core (256)
TB = 256         # token block for streaming K/V projections
NTB = NK // TB   # 16
TCH = NK // 128  # 32 t-chunks for attention
QT = 512         # q tile
NJ = NQ // QT    # 4
SHIFT = 2.5      # softmax exp shift (softmax-invariant)

# Schraudolph exp approximation on DVE: int16 bit pattern of bf16.
# exp(s/8 - SHIFT) ~= bf16_bits( s * A + B ), C=7 calibrated for zero bias.
SCH_A = float(128.0 * np.log2(np.e) / 8.0)
SCH_B = float(127 * 128 - 7.0 - 128.0 * np.log2(np.e) * SHIFT)


def _sch_p1(t):
    # phase-1 chased sweeps: DMA-bound, ScalarE has slack
    return 'act'


_P2_DVE = frozenset((1, 3, 5, 7, 9, 11, 13, 15, 17, 19, 21, 23, 25, 27, 29, 31))


def _sch_p2(t):
    # phase 2: 16/32 of chunks on the DVE bit-trick exp
    return 'dve' if t in _P2_DVE else 'act'


_last_results = None
_last_in_maps = None


def _build():
    nc = bacc.Bacc("TRN2", target_bir_lowering=False, debug=False, num_devices=8)

    keyT_d = nc.dram_tensor("keyT", [E, NK], F32, kind="ExternalInput").ap()
    valT_d = nc.dram_tensor("valT", [E, NK], F32, kind="ExternalInput").ap()
    qT_d = nc.dram_tensor("qT", [128, 2, NQ], F32, kind="ExternalInput").ap()
    wkT_d = nc.dram_tensor("wkT", [128, 8, 2, 128], F32, kind="ExternalInput").ap()
    wvT_d = nc.dram_tensor("wvT", [E, C], F32, kind="ExternalInput").ap()
    woT_d = nc.dram_tensor("woT", [128, 2, E], F32, kind="ExternalInput").ap()
    ident_d = nc.dram_tensor("ident", [128, 128], BF16, kind="ExternalInput").ap()
    out_d = nc.dram_tensor("out", [NQ, E], BF16, kind="ExternalOutput").ap()

    with tile.TileContext(nc) as tc:
        with (
            tc.tile_pool(name="wpool", bufs=1) as wpool,
            tc.tile_pool(name="stream", bufs=4) as stream,
            tc.tile_pool(name="big", bufs=1) as big,
            tc.tile_pool(name="expp", bufs=8) as expp,
            tc.tile_pool(name="epil", bufs=3) as epil,
            tc.tile_pool(name="nmp", bufs=3) as nmp,
            tc.tile_pool(name="outsb", bufs=3) as outsb,
        ):
            # ---- resident weights / q / identity ----
            wk_sb = wpool.tile([128, 8, 2, 128], F32R)
            wv_sb = wpool.tile([128, 8, C], F32R)
            wo_f32 = wpool.tile([128, 2, E], F32)
            wo_sb = wpool.tile([128, 2, E], BF16)
            q_sb = wpool.tile([128, 2, NQ], F32R)
            id_sb = wpool.tile([128, 128], BF16)
            bias_sb = wpool.tile([128, 1], F32)
            nc.vector.memset(bias_sb[:], -SHIFT)
            nc.sync.dma_start(wk_sb[:], wkT_d[:].bitcast(F32R))

            # ---- resident kT (fp32) / vx (bf16, 65th col = ones) ----
            kT_sb = big.tile([128, 2, NK], F32R)            # [(h%2)*64+d, h//2, t]
            vx_sb = big.tile([128, TCH, HPC, D + 1], BF16)  # [t%128, t//128, h, d|1]
            nc.vector.memset(vx_sb[:, :, :, D:D + 1], 1.0)

            def emit_score_chunk(sc, h, t, q0):
                # sc [128,512] <- scores of one 128-token chunk for head h
                hp, mc = h % 2, h // 2
                nc.tensor.matmul(
                    sc[:],
                    kT_sb[hp * 64:(hp + 1) * 64, mc, t * 128:(t + 1) * 128],
                    q_sb[hp * 64:(hp + 1) * 64, mc, q0:q0 + QT],
                    start=True, stop=True, tile_position=(hp * 64, 0))

            def emit_exp(ex, sc, eng):
                # ex [128,512] bf16 <- exp(sc/8 - SHIFT)
                if eng == 'act':
                    nc.scalar.activation(ex[:], sc[:], EXPF, bias=bias_sb[:], scale=0.125)
                else:
                    e = nc.vector if eng == 'dve' else nc.gpsimd
                    e.tensor_scalar(
                        ex[:].bitcast(I16), sc[:], SCH_A, SCH_B,
                        mybir.AluOpType.mult, mybir.AluOpType.add)

            def emit_attnv(oT, ex, h, t, first, last):
                # transposed attn@v into one psum bank [128, 4(qc), 128]:
                # oT[:, qc, 0:65] += ex[:, qslice].T @ vx[:, t, h, :]
                for qc in range(4):
                    nc.tensor.matmul(
                        oT[:, qc, 0:D + 1],
                        ex[:, qc * 128:(qc + 1) * 128],
                        vx_sb[:, t, h, :],
                        start=(first and qc == 0),
                        stop=(last and qc == 3),
                        skip_group_check=True)

            def emit_chunk(ppool, oT, h, t, q0, first, last, eng):
                sc = ppool.tile([128, QT], F32, tag="sc", name="sc")
                emit_score_chunk(sc, h, t, q0)
                ex = expp.tile([128, QT], BF16, tag="ex", name="ex")
                emit_exp(ex, sc, eng)
                emit_attnv(oT, ex, h, t, first, last)

            def emit_pair(ppool, oT, h, tp, q0, eng):
                # two t-chunks per psum tile -> one double-width exp instr
                sc = ppool.tile([128, 2, QT], F32, tag="sc2", name="sc2")
                for i in range(2):
                    emit_score_chunk(sc[:, i, :], h, 2 * tp + i, q0)
                ex = expp.tile([128, 2, QT], BF16, tag="ex2", name="ex2")
                emit_exp(ex, sc, eng)
                for i in range(2):
                    t = 2 * tp + i
                    emit_attnv(oT, ex[:, i, :], h, t, t == 0, t == TCH - 1)

            def emit_epilogue(oT, nm_sb, h):
                # per-partition normalization: denominator at free col 64
                rc = epil.tile([128, HPC, 1], F32R, tag="rc", name="rc")
                with nc.allow_low_precision(reason="f32r recip, ~19-bit mantissa is ample"):
                    nc.vector.reciprocal(rc[:], oT[:, :, D:D + 1])
                nc.vector.tensor_tensor(
                    nm_sb[:, :, h, :], oT[:, :, 0:D],
                    rc[:].bitcast(F32).broadcast_to([128, HPC, D]),
                    mybir.AluOpType.mult)

            def emit_tp(nmT, nm_sb, pr, ppool, dve_copy=False):
                # PE-transpose of one head-pair of nm (emitted as soon as
                # both heads' epilogues are done, overlapping later sweeps).
                # Scratch is a bitcast view of a W_o-ring tile.
                w = ppool.tile([128, QT], F32, tag="wps", name="tp")
                tp = w[:, 0:256].bitcast(BF16).rearrange("p (qc n) -> p qc n", qc=4)
                for qc in range(4):
                    nc.tensor.matmul(tp[:, qc, :],
                                     nm_sb[:, qc, 2 * pr:2 * pr + 2, :],
                                     id_sb[:], is_transpose=True)
                cp = nc.vector.tensor_copy if dve_copy else nc.scalar.copy
                cp(nmT[:, pr, :].rearrange("p (qc n) -> p qc n", qc=4), tp[:])

            def emit_wo_items(j, nmT, ppool, tag="wps", last=False):
                # W_o as 8 small work items ((qc, et) granularity) so they can
                # interleave with the next sweep's chunks on the PE
                q0 = j * QT
                items = []
                for qc in range(4):
                    box = {}
                    for et in range(2):
                        def item(qc=qc, et=et, box=box):
                            if et == 0:
                                box['osb'] = outsb.tile([128, E], BF16, tag="osb", name="osb")
                            osb = box['osb']
                            wps = ppool.tile([128, QT], F32, tag=tag, name="wps")
                            for pr in range(2):
                                nc.tensor.matmul(wps[:],
                                                 nmT[:, pr, qc * 128:(qc + 1) * 128],
                                                 wo_sb[:, pr, et * QT:(et + 1) * QT],
                                                 start=(pr == 0), stop=(pr == 1))
                            if et == 0 or last:
                                nc.vector.tensor_copy(osb[:, et * QT:(et + 1) * QT], wps[:])
                            else:
                                nc.scalar.copy(osb[:, QT:2 * QT], wps[:])
                            nc.sync.dma_start(
                                out_d[q0 + qc * 128:q0 + (qc + 1) * 128, et * QT:(et + 1) * QT],
                                osb[:, et * QT:(et + 1) * QT])
                        items.append(item)
                return items

            # ============ phase 1: stream K/V + projections + chase ============
            # three j0 sweeps chase the streamed chunks
            NCH = 3
            nm_j0 = nmp.tile([128, HPC, HPC, D], BF16, tag="nm", name="nmj0")
            with (
                tc.tile_pool(name="pproj", bufs=3, space="PSUM") as pproj,
                tc.tile_pool(name="ppair", bufs=2, space="PSUM") as ppair,
                tc.tile_pool(name="pout1", bufs=3, space="PSUM") as pout1,
            ):
                oT_chase = {h: pout1.tile([128, HPC, 128], F32, tag="oT1", name="oT1")
                            for h in range(NCH)}
                for tb in range(NTB):
                    kblk = stream.tile([128, 8, TB], F32R, tag="kblk", name="kblk")
                    vblk = stream.tile([128, 8, TB], F32R, tag="vblk", name="vblk")
                    ts0 = tb * TB
                    nc.sync.dma_start(kblk[:], keyT_d.rearrange("(c p) t -> p c t", p=128)[:, :, ts0:ts0 + TB].bitcast(F32R))
                    if tb == 0:
                        nc.sync.dma_start(q_sb[:, :, 0:QT], qT_d[:, :, 0:QT].bitcast(F32R))
                    nc.sync.dma_start(vblk[:], valT_d.rearrange("(c p) t -> p c t", p=128)[:, :, ts0:ts0 + TB].bitcast(F32R))
                    if tb == 0:
                        nc.sync.dma_start(wv_sb[:], wvT_d.rearrange("(c p) n -> p c n", p=128).bitcast(F32R))
                    # K projection: two column halves -> kT partitions
                    for mc in range(2):
                        kps = pproj.tile([128, TB], F32, tag="pp", name="kps")
                        for c in range(8):
                            nc.tensor.matmul(kps[:], wk_sb[:, c, mc, :],
                                             kblk[:, c, :], start=(c == 0), stop=(c == 7))
                        nc.vector.tensor_copy(kT_sb[:, mc, ts0:ts0 + TB], kps[:])
                    # V projection -> bf16 vx (ones column untouched)
                    for t2 in range(TB // 128):
                        vps = pproj.tile([128, C], F32, tag="pp", name="vps")
                        for c in range(8):
                            nc.tensor.matmul(vps[:], vblk[:, c, t2 * 128:(t2 + 1) * 128],
                                             wv_sb[:, c, :], start=(c == 0), stop=(c == 7))
                        tg = tb * (TB // 128) + t2
                        nc.vector.tensor_copy(
                            vx_sb[:, tg, :, 0:D],
                            vps[:].rearrange("p (h d) -> p h d", h=HPC))
                    # chase: j0 sweeps on this block's two chunks
                    for h in range(NCH):
                        for i in range(2):
                            t = 2 * tb + i
                            emit_chunk(ppair, oT_chase[h], h, t, 0,
                                       t == 0, t == TCH - 1, _sch_p1(t))
                # late, non-critical loads: q tail (j1+), W_o, identity
                nc.sync.dma_start(q_sb[:, :, QT:NQ], qT_d[:, :, QT:NQ].bitcast(F32R))
                nc.sync.dma_start(wo_f32[:], woT_d[:])
                nc.sync.dma_start(id_sb[:], ident_d[:])
                nc.vector.tensor_copy(wo_sb[:], wo_f32[:])
                for h in range(NCH):
                    emit_epilogue(oT_chase[h], nm_j0, h)

            # ============ phase 2: remaining sweeps + all W_o ============
            with (
                tc.tile_pool(name="ppair2", bufs=4, space="PSUM") as ppair2,
                tc.tile_pool(name="pout2", bufs=2, space="PSUM") as pout2,
                tc.tile_pool(name="psw", bufs=2, space="PSUM") as psw,
            ):
                def sweep(j, h, nm_sb, piggy=None):
                    oT = pout2.tile([128, HPC, 128], F32, tag="oT2", name="oT2")
                    for t in range(TCH):
                        emit_chunk(ppair2, oT, h, t, j * QT, t == 0, t == TCH - 1,
                                   _sch_p2(t))
                        if piggy:
                            piggy.pop(0)()
                    while piggy:
                        piggy.pop(0)()
                    emit_epilogue(oT, nm_sb, h)

                # finish j0: pair-0 transpose, head-3 sweep
                nmT0 = nmp.tile([128, 2, QT], BF16, tag="nmT", name="nmT")
                emit_tp(nmT0, nm_j0, 0, psw)
                sweep(0, 3, nm_j0)
                emit_tp(nmT0, nm_j0, 1, psw)
                pending = emit_wo_items(0, nmT0, psw)
                for j in range(1, NJ):
                    nm_sb = nmp.tile([128, HPC, HPC, D], BF16, tag="nm", name="nm")
                    nmT = nmp.tile([128, 2, QT], BF16, tag="nmT", name="nmT")
                    for h in range(HPC):
                        sweep(j, h, nm_sb, piggy=pending if h == 0 else None)
                        pending = None
                        if h == 1:
                            emit_tp(nmT, nm_sb, 0, psw)
                    emit_tp(nmT, nm_sb, 1, psw, dve_copy=(j == NJ - 1))
                    # the last j's items flush serially at the end; give
                    # them the score ring (idle by then) for more overlap
                    if j == NJ - 1:
                        pending = emit_wo_items(j, nmT, ppair2, tag="sc", last=True)
                    else:
                        pending = emit_wo_items(j, nmT, psw)
                # last j: no next sweep to ride on
                for it in pending:
                    it()

    nc.compile()
    return nc


_nc = None


def kernel(query, key, value, W_k, W_v, W_o):
    global _nc, _last_results, _last_in_maps
    if _nc is None:
        _nc = _build()

    query = np.asarray(query, dtype=np.float32)
    key = np.asarray(key, dtype=np.float32)
    value = np.asarray(value, dtype=np.float32)
    W_k = np.asarray(W_k, dtype=np.float32)
    W_v = np.asarray(W_v, dtype=np.float32)
    W_o = np.asarray(W_o, dtype=np.float32)

    import ml_dtypes
    ident = np.eye(128, dtype=ml_dtypes.bfloat16)
    keyT = [np.ascontiguousarray(key[b].T) for b in range(B)]
    valT = [np.ascontiguousarray(value[b].T) for b in range(B)]

    in_maps = []
    for b in range(B):
        for g in range(4):
            c0 = g * C
            # channel c_core = h*64+d -> (mc=h//2, p=(h%2)*64+d)
            qg = query[b][:, c0:c0 + C].T.reshape(HPC, D, NQ)  # [h, d, n]
            qT = np.empty((128, 2, NQ), np.float32)
            wk = W_k[c0:c0 + C, :].reshape(HPC, D, E)          # [h, d, e]
            wkT = np.empty((128, 8, 2, 128), np.float32)       # [e%128, e//128, mc, p]
            for h in range(HPC):
                hp, mc = h % 2, h // 2
                qT[hp * 64:(hp + 1) * 64, mc] = qg[h]
                wkT[:, :, mc, hp * 64:(hp + 1) * 64] = (
                    wk[h].T.reshape(8, 128, D).transpose(1, 0, 2))
            # wo rows p2 = (h%2)*64+d, pr = h//2  (c_core = pr*128 + p2)
            woT = np.ascontiguousarray(
                W_o[:, c0:c0 + C].T.reshape(2, 128, E).transpose(1, 0, 2))
            in_maps.append({
                "keyT": keyT[b],
                "valT": valT[b],
                "qT": np.ascontiguousarray(qT),
                "wkT": np.ascontiguousarray(wkT),
                "wvT": np.ascontiguousarray(W_v[c0:c0 + C, :].T),
                "woT": woT,
                "ident": ident,
            })

    _last_in_maps = in_maps
    res = run_bass_kernel_spmd(
        _nc, in_maps, core_ids=list(range(8)),
        trace=bool(os.environ.get("BASS_TRACE")))
    _last_results = res

    out = np.zeros((B, NQ, E), dtype=np.float32)
    for b in range(B):
        for g in range(4):
            out[b] += res.results[b * 4 + g]["out"].astype(np.float32)
    return out
